# revision 2
# baseline (speedup 1.0000x reference)
"""Pre-Norm + GQA + Gain + Residual for TRN2, 8 NeuronCores.

Sharding: sequence-parallel. 8 cores = 4 batch elements x 2 query-chunk
sets. The 2048-token sequence is split into four 512-token chunks; causal
work per chunk is proportional to (1+chunk index), so variant-A cores own
chunks (0,3) and variant-B cores own chunks (1,2) — both 50% of the work.
Each core computes K/V for its full batch element (duplicated across the
pair), all 20 heads for its own query rows, and full output rows for those
queries — no collectives. Two SPMD programs (A on cores 0-3, B on 4-7)
are dispatched concurrently on disjoint device subsets.

Kernel numerics: bf16 matmuls with fp32 accumulation everywhere; softmax
without max-subtraction (scores are bounded ~|6| for this distribution);
softmax denominator from an appended ones-column in V; RMS-norm folded into
RoPE/V-scaling (rstd) and weights (norm_w); attention scale folded into wq;
gain folded into wo.
"""

import math
import os

os.environ.setdefault("NEURON_COMPILE_CACHE_URL", "/root/neuron_cache")

from contextlib import ExitStack

import numpy as np
import ml_dtypes

import concourse.bass as bass
import concourse.bacc as bacc
import concourse.tile as tile
from concourse import mybir
from concourse import bass2jax
from concourse.masks import make_identity

F32 = mybir.dt.float32
BF16 = mybir.dt.bfloat16
TS = 128

B, S, D = 4, 2048, 1280
NH, NKV, HD = 20, 4, 64
CHUNK = 512
EPS = 1e-5
NUM_LAYERS = 20
CHUNKS_A = (0, 3)
CHUNKS_B = (1, 2)


def _col_groups(total, width=512):
    out, o = [], 0
    while o < total:
        w = min(width, total - o)
        out.append((o, w))
        o += w
    return out


def build_program(chunks):
    NT = S // TS
    DCH = D // 128
    CT = CHUNK // TS
    NPAIR = NH // 2
    GQ = NH // NKV
    HALF = HD // 2
    DQ, DKV = NH * HD, NKV * HD
    OWN = list(chunks)
    SQ_OWN = len(OWN) * CHUNK
    EGQ = _col_groups(DQ)
    EGO = _col_groups(D)
    KCH = DKV // 128

    nc = bacc.Bacc("TRN2", target_bir_lowering=False, debug=False)

    x = nc.dram_tensor("x", [S, D], F32, kind="ExternalInput").ap()
    resid = nc.dram_tensor("resid", [SQ_OWN, D], F32, kind="ExternalInput").ap()
    wqT = nc.dram_tensor("wqT", [D, DQ], BF16, kind="ExternalInput").ap()
    wkT = nc.dram_tensor("wkT", [D, DKV], BF16, kind="ExternalInput").ap()
    wvT = nc.dram_tensor("wvT", [D, DKV], BF16, kind="ExternalInput").ap()
    woT = nc.dram_tensor("woT", [D, D], BF16, kind="ExternalInput").ap()
    cosS = nc.dram_tensor("cosS", [S, HALF], F32, kind="ExternalInput").ap()
    sinS = nc.dram_tensor("sinS", [S, HALF], F32, kind="ExternalInput").ap()
    out = nc.dram_tensor("out", [SQ_OWN, D], F32, kind="ExternalOutput").ap()

    with tile.TileContext(nc) as tc, ExitStack() as ctx:
        P = lambda **kw: ctx.enter_context(tc.tile_pool(**kw))
        p_w = P(name="w", bufs=DCH)
        p_wk = P(name="wk", bufs=DCH)
        p_wv = P(name="wv", bufs=DCH)
        p_x = P(name="x", bufs=2)
        p_xb = P(name="xb", bufs=CT + 1)
        p_sq = P(name="sq", bufs=1)
        p_t512 = P(name="t512", bufs=2 * DCH + 2)
        p_qro = P(name="qro", bufs=CT + 1)
        p_kro = P(name="kro", bufs=CT + 1)
        p_qt = P(name="qt", bufs=len(OWN) * NPAIR)
        p_kt = P(name="kt", bufs=NKV)
        p_v = P(name="v", bufs=NT)
        p_cs = P(name="cs", bufs=4)
        p_exp = P(name="exp", bufs=4)
        p_rt = P(name="rt", bufs=6)
        p_odd = P(name="odd", bufs=2)
        p_ro = P(name="ro", bufs=2)
        p_sm = P(name="sm", bufs=1)
        p_rc = P(name="rc", bufs=3)
        p_rcb = P(name="rcb", bufs=2)

        # PSUM: 8 banks total; 5-slot pool (projections/scores/o-proj) +
        # 2-slot pool (transpose batches / PV accumulators), tags reused
        # across phases.
        psA = P(name="psA", bufs=5, space="PSUM")
        psB = P(name="psB", bufs=2, space="PSUM")

        ident = p_sm.tile([128, 128], BF16, tag="ident")
        make_identity(nc, ident)
        msall = p_sm.tile([128, NT], F32, tag="msall")
        srtall = p_sm.tile([128, NT], F32, tag="srtall")
        rstdall = p_sm.tile([128, NT], F32, tag="rstdall")
        epsb = p_sm.tile([128, 1], F32, tag="epsb")
        nc.vector.memset(epsb, float(EPS))

        wq_t, wk_t, wv_t = [], [], []
        for c in range(DCH):
            t = p_w.tile([128, DQ], BF16, tag="w")
            nc.sync.dma_start(out=t, in_=wqT[c * 128:(c + 1) * 128, :])
            wq_t.append(t)
            tk = p_wk.tile([128, DKV], BF16, tag="wk")
            nc.sync.dma_start(out=tk, in_=wkT[c * 128:(c + 1) * 128, :])
            wk_t.append(tk)
            tv = p_wv.tile([128, DKV], BF16, tag="wv")
            nc.sync.dma_start(out=tv, in_=wvT[c * 128:(c + 1) * 128, :])
            wv_t.append(tv)

        kTr = [p_kt.tile([128, S], BF16, tag="kt", name=f"kTr{i}")
               for i in range(NKV)]
        V4 = [p_v.tile([128, NKV, HD + 1], BF16, tag="v", name=f"V4_{i}")
              for i in range(NT)]
        qT = {}
        for ci in range(len(OWN)):
            for p in range(NPAIR):
                qT[(ci, p)] = p_qt.tile([128, CHUNK], BF16, tag="qt",
                                        name=f"qT_{ci}_{p}")

        def view3(ap, off, mid, inner):
            a = ap[:] if not isinstance(ap, bass.AP) else ap
            return bass.AP(tensor=a.tensor, offset=a.offset + off,
                           ap=[list(a.ap[0]), list(mid), list(inner)])

        # ================= PHASE 1 =================
        for g in range(NT // CT):
            s0 = g * CT * TS
            own_ci = OWN.index(g) if g in OWN else None
            x_b, kro_g, qro_g = [], [], []

            for tl in range(CT):
                t = g * CT + tl
                xt = p_x.tile([128, D], F32, tag="x")
                nc.sync.dma_start(out=xt,
                                  in_=x[s0 + tl * TS:s0 + (tl + 1) * TS, :])
                sqs = p_sq.tile([128, D], BF16, tag="sq")
                nc.scalar.activation(
                    out=sqs, in_=xt, func=mybir.ActivationFunctionType.Square,
                    accum_out=msall[:, t:t + 1])
                nc.scalar.activation(
                    out=srtall[:, t:t + 1], in_=msall[:, t:t + 1],
                    func=mybir.ActivationFunctionType.Sqrt,
                    scale=1.0 / D, bias=epsb[:])
                nc.vector.reciprocal(out=rstdall[:, t:t + 1],
                                     in_=srtall[:, t:t + 1])
                xb = p_xb.tile([128, D], BF16, tag="xb")
                nc.vector.tensor_copy(out=xb, in_=xt)
                x_b.append(xb)

            xT_g = []
            for c in range(DCH):
                pst = psB.tile([128, CT * TS], BF16, tag="b")
                for tl in range(CT):
                    nc.tensor.transpose(
                        out=pst[:, tl * TS:(tl + 1) * TS],
                        in_=x_b[tl][:, c * 128:(c + 1) * 128], identity=ident)
                xts = p_t512.tile([128, CT * TS], BF16, tag="t512")
                nc.vector.tensor_copy(out=xts, in_=pst)
                xT_g.append(xts)

            for tl in range(CT):
                t = g * CT + tl
                rstd = rstdall[:, t:t + 1]
                cs_t = p_cs.tile([128, HALF], F32, tag="cs")
                nc.sync.dma_start(out=cs_t, in_=cosS[t * TS:(t + 1) * TS, :])
                sn_t = p_cs.tile([128, HALF], F32, tag="cs")
                nc.sync.dma_start(out=sn_t, in_=sinS[t * TS:(t + 1) * TS, :])

                def rope(ps, nheads, out_tile, out_off):
                    csb = view3(cs_t, 0, (0, nheads), (1, HALF))
                    snb = view3(sn_t, 0, (0, nheads), (1, HALF))
                    x1 = view3(ps, 0, (HD, nheads), (1, HALF))
                    x2 = view3(ps, HALF, (HD, nheads), (1, HALF))
                    o1 = view3(out_tile, out_off, (HD, nheads), (1, HALF))
                    o2 = view3(out_tile, out_off + HALF, (HD, nheads),
                               (1, HALF))
                    M = mybir.AluOpType.mult
                    tmp = [p_rt.tile([128, nheads * HALF], F32, tag="rt",
                                     name=f"rt{i}") for i in range(4)]
                    nc.vector.scalar_tensor_tensor(
                        out=tmp[0], in0=x1, scalar=rstd, in1=csb, op0=M, op1=M)
                    nc.vector.scalar_tensor_tensor(
                        out=tmp[1], in0=x2, scalar=rstd, in1=snb, op0=M, op1=M)
                    nc.vector.scalar_tensor_tensor(
                        out=tmp[2], in0=x1, scalar=rstd, in1=snb, op0=M, op1=M)
                    nc.vector.scalar_tensor_tensor(
                        out=tmp[3], in0=x2, scalar=rstd, in1=csb, op0=M, op1=M)
                    nc.vector.tensor_sub(out=o1, in0=tmp[0], in1=tmp[1])
                    nc.vector.tensor_add(out=o2, in0=tmp[2], in1=tmp[3])

                psk = psA.tile([128, DKV], F32, tag="a")
                for c in range(DCH):
                    nc.tensor.matmul(
                        psk, lhsT=xT_g[c][:, tl * TS:(tl + 1) * TS],
                        rhs=wk_t[c], start=(c == 0), stop=(c == DCH - 1))
                kro = p_kro.tile([128, DKV], BF16, tag="kro")
                rope(psk, NKV, kro, 0)
                kro_g.append(kro)

                psv = psA.tile([128, DKV], F32, tag="a")
                for c in range(DCH):
                    nc.tensor.matmul(
                        psv, lhsT=xT_g[c][:, tl * TS:(tl + 1) * TS],
                        rhs=wv_t[c], start=(c == 0), stop=(c == DCH - 1))
                nc.vector.tensor_scalar_mul(
                    out=V4[t][:, :, 0:HD],
                    in0=view3(psv, 0, (HD, NKV), (1, HD)), scalar1=rstd)
                nc.vector.memset(V4[t][:, :, HD:HD + 1], 1.0)

                if own_ci is not None:
                    qro = p_qro.tile([128, DQ], BF16, tag="qro")
                    for (eo, ew) in EGQ:
                        psq = psA.tile([128, ew], F32, tag="a")
                        for c in range(DCH):
                            nc.tensor.matmul(
                                psq, lhsT=xT_g[c][:, tl * TS:(tl + 1) * TS],
                                rhs=wq_t[c][:, eo:eo + ew],
                                start=(c == 0), stop=(c == DCH - 1))
                        rope(psq, ew // HD, qro, eo)
                    qro_g.append(qro)

            for c in range(KCH):
                pst = psB.tile([128, CT * TS], BF16, tag="b")
                for tl in range(CT):
                    nc.tensor.transpose(
                        out=pst[:, tl * TS:(tl + 1) * TS],
                        in_=kro_g[tl][:, c * 128:(c + 1) * 128],
                        identity=ident)
                ge, go = 2 * c, 2 * c + 1
                cols = slice(s0, s0 + CT * TS)
                nc.vector.tensor_copy(out=kTr[ge][0:64, cols],
                                      in_=pst[0:64, :])
                nc.vector.tensor_copy(out=kTr[go][64:128, cols],
                                      in_=pst[64:128, :])
                nc.sync.dma_start(out=kTr[ge][64:128, cols],
                                  in_=kTr[ge][0:64, cols])
                nc.sync.dma_start(out=kTr[go][0:64, cols],
                                  in_=kTr[go][64:128, cols])

            if own_ci is not None:
                for p in range(NPAIR):
                    pst = psB.tile([128, CT * TS], BF16, tag="b")
                    for tl in range(CT):
                        nc.tensor.transpose(
                            out=pst[:, tl * TS:(tl + 1) * TS],
                            in_=qro_g[tl][:, p * 128:(p + 1) * 128],
                            identity=ident)
                    nc.vector.tensor_copy(out=qT[(own_ci, p)], in_=pst)

        # ================= ATTENTION =================
        attnT = {}
        wo_t = None
        for ci, a in enumerate(OWN):
            for p in range(NPAIR):
                attnT[(ci, p)] = p_t512.tile([128, CHUNK], BF16, tag="t512",
                                             name=f"attnT_{ci}_{p}")
            nkt = (a + 1) * CT
            for h in range(NH):
                p, half = h // 2, h % 2
                base = half * 64
                g = h // GQ
                pv = psB.tile([HD + 1, CHUNK], F32, tag="b")
                for kt in range(nkt):
                    off = max(0, (kt - a * CT)) * TS
                    N = CHUNK - off
                    pss = psA.tile([128, N], F32, tag="a")
                    nc.tensor.matmul(
                        pss,
                        lhsT=kTr[g][base:base + 64, kt * TS:(kt + 1) * TS],
                        rhs=qT[(ci, p)][base:base + 64, off:CHUNK],
                        start=True, stop=True)
                    ex = p_exp.tile([128, N], BF16, tag="exp")
                    nc.scalar.activation(
                        out=ex, in_=pss,
                        func=mybir.ActivationFunctionType.Exp)
                    if kt >= a * CT:
                        nc.gpsimd.affine_select(
                            out=ex[:, 0:TS], in_=ex[:, 0:TS],
                            compare_op=mybir.AluOpType.is_ge, fill=0.0,
                            base=0, pattern=[[1, TS]], channel_multiplier=-1)
                    nc.tensor.matmul(
                        pv[:, off:CHUNK], lhsT=V4[kt][:, g, :], rhs=ex,
                        start=(kt == 0), stop=(kt == nkt - 1),
                        skip_group_check=True)
                rc = p_rc.tile([1, CHUNK], F32, tag="rc")
                nc.vector.reciprocal(out=rc, in_=pv[HD:HD + 1, :])
                rcb_t = p_rcb.tile([64, CHUNK], F32, tag="rcb")
                nc.gpsimd.partition_broadcast(rcb_t[:], rc[:])
                if half == 0:
                    nc.vector.tensor_mul(
                        out=attnT[(ci, p)][0:64, :], in0=pv[0:HD, :],
                        in1=rcb_t[:])
                else:
                    stg = p_odd.tile([64, CHUNK], BF16, tag="odd")
                    nc.vector.tensor_mul(out=stg, in0=pv[0:HD, :],
                                         in1=rcb_t[:])
                    nc.sync.dma_start(out=attnT[(ci, p)][64:128, :], in_=stg)

            if wo_t is None:
                wo_t = []
                for c in range(DCH):
                    t = p_w.tile([128, D], BF16, tag="w")
                    nc.sync.dma_start(out=t,
                                      in_=woT[c * 128:(c + 1) * 128, :])
                    wo_t.append(t)
            for sb in range(CT):
                rowp = ci * CHUNK + sb * TS
                rt = p_ro.tile([128, D], F32, tag="resid")
                nc.sync.dma_start(out=rt, in_=resid[rowp:rowp + TS, :])
                ot = p_ro.tile([128, D], F32, tag="out")
                for (eo, ew) in EGO:
                    pso = psA.tile([128, ew], F32, tag="a")
                    for c in range(DCH):
                        nc.tensor.matmul(
                            pso,
                            lhsT=attnT[(ci, c)][:, sb * TS:(sb + 1) * TS],
                            rhs=wo_t[c][:, eo:eo + ew],
                            start=(c == 0), stop=(c == DCH - 1))
                    nc.vector.tensor_add(
                        out=ot[:, eo:eo + ew], in0=pso, in1=rt[:, eo:eo + ew])
                nc.sync.dma_start(out=out[rowp:rowp + TS, :], in_=ot)

    nc.compile()
    return nc


# ====================== host-side prep ======================

def _rope_perm(nheads, hd):
    idx = []
    for h in range(nheads):
        idx.extend(h * hd + np.arange(0, hd, 2))
        idx.extend(h * hd + np.arange(1, hd, 2))
    return np.asarray(idx)


def _prepare_weights(wq, wk, wv, wo, norm_w, gain):
    bf16 = ml_dtypes.bfloat16
    scale = 1.0 / math.sqrt(HD)
    wqT = ((wq * norm_w[None, :] * scale).T)[:, _rope_perm(NH, HD)]
    wkT = ((wk * norm_w[None, :]).T)[:, _rope_perm(NKV, HD)]
    wvT = (wv * norm_w[None, :]).T
    woT = (wo * float(gain)).T
    pos = np.arange(S, dtype=np.float64)[:, None]
    dim = np.arange(0, HD, 2, dtype=np.float64)[None, :]
    freqs = pos / (10000.0 ** (dim / HD))
    return (np.ascontiguousarray(wqT).astype(bf16),
            np.ascontiguousarray(wkT).astype(bf16),
            np.ascontiguousarray(wvT).astype(bf16),
            np.ascontiguousarray(woT).astype(bf16),
            np.cos(freqs).astype(np.float32),
            np.sin(freqs).astype(np.float32))


def _core_inputs(xb, residb, W, chunks):
    wqT, wkT, wvT, woT, cosS, sinS = W
    rows = np.concatenate(
        [np.arange(a * CHUNK, (a + 1) * CHUNK) for a in chunks])
    return {
        "x": np.ascontiguousarray(xb, dtype=np.float32),
        "resid": np.ascontiguousarray(residb[rows]).astype(np.float32),
        "wqT": wqT, "wkT": wkT, "wvT": wvT, "woT": woT,
        "cosS": cosS, "sinS": sinS,
    }


# ====================== dispatch ======================

_CACHE = {}


def _make_runner(nc, devices):
    """Device-subset variant of bass2jax.run_bass_via_pjrt's multi-core path."""
    import jax
    from jax.experimental.shard_map import shard_map
    from jax.sharding import Mesh, PartitionSpec

    bass2jax.install_neuronx_cc_hook()
    n_cores = len(devices)
    part_name = (nc.partition_id_tensor.name
                 if nc.partition_id_tensor is not None else None)

    in_names, out_names, out_avals, zero_outs = [], [], [], []
    for alloc in nc.m.functions[0].allocations:
        if not isinstance(alloc, mybir.MemoryLocationSet):
            continue
        name = alloc.memorylocations[0].name
        if alloc.kind == "ExternalInput":
            if name != part_name:
                in_names.append(name)
        elif alloc.kind == "ExternalOutput":
            out_names.append(name)
            shape = tuple(alloc.tensor_shape)
            dtype = mybir.dt.np(alloc.dtype)
            out_avals.append(jax.core.ShapedArray(shape, dtype))
            zero_outs.append(np.zeros(shape, dtype))
    n_params = len(in_names)
    all_names = in_names + out_names
    if part_name is not None:
        all_names = all_names + [part_name]
    donate = tuple(range(n_params, n_params + len(out_names)))

    def _body(*args):
        operands = list(args)
        if part_name is not None:
            operands.append(bass2jax.partition_id_tensor())
        outs = bass2jax._bass_exec_p.bind(
            *operands,
            out_avals=tuple(out_avals),
            in_names=tuple(all_names),
            out_names=tuple(out_names),
            lowering_input_output_aliases=(),
            sim_require_finite=True,
            sim_require_nnan=True,
            nc=nc,
        )
        return tuple(outs)

    mesh = Mesh(np.asarray(devices), ("core",))
    nio = n_params + len(out_names)
    sharded = jax.jit(
        shard_map(_body, mesh=mesh,
                  in_specs=(PartitionSpec("core"),) * nio,
                  out_specs=(PartitionSpec("core"),) * len(out_names),
                  check_rep=False),
        donate_argnums=donate, keep_unused=True)

    def run(in_maps):
        per_core = [[np.asarray(m[k]) for k in in_names] for m in in_maps]
        concat_in = [
            np.concatenate([per_core[c][i] for c in range(n_cores)], axis=0)
            for i in range(n_params)]
        concat_zeros = [
            np.zeros((n_cores * z.shape[0], *z.shape[1:]), z.dtype)
            for z in zero_outs]
        return sharded(*concat_in, *concat_zeros)  # async jax arrays

    def collect(out_arrs):
        return [
            {name: np.asarray(out_arrs[i]).reshape(
                n_cores, *out_avals[i].shape)[c]
             for i, name in enumerate(out_names)}
            for c in range(n_cores)]

    return run, collect


def _get_runners():
    if "runners" not in _CACHE:
        import jax
        devs = jax.devices()
        assert len(devs) >= 8, f"need 8 neuron cores, have {len(devs)}"
        nc_a = build_program(CHUNKS_A)
        nc_b = build_program(CHUNKS_B)
        run_a, col_a = _make_runner(nc_a, devs[0:4])
        run_b, col_b = _make_runner(nc_b, devs[4:8])
        _CACHE["runners"] = (run_a, col_a, run_b, col_b)
    return _CACHE["runners"]


def kernel(x, residual, norm_w, wq, wk, wv, wo, gain):
    x = np.asarray(x, np.float32)
    residual = np.asarray(residual, np.float32)
    W = _prepare_weights(np.asarray(wq, np.float32),
                         np.asarray(wk, np.float32),
                         np.asarray(wv, np.float32),
                         np.asarray(wo, np.float32),
                         np.asarray(norm_w, np.float32),
                         np.asarray(gain, np.float32))

    run_a, col_a, run_b, col_b = _get_runners()
    maps_a = [_core_inputs(x[b], residual[b], W, CHUNKS_A) for b in range(B)]
    maps_b = [_core_inputs(x[b], residual[b], W, CHUNKS_B) for b in range(B)]

    arrs_a = run_a(maps_a)   # async dispatch on cores 0-3
    arrs_b = run_b(maps_b)   # async dispatch on cores 4-7
    res_a = col_a(arrs_a)
    res_b = col_b(arrs_b)

    out = np.empty((B, S, D), np.float32)
    for b in range(B):
        for i, a in enumerate(CHUNKS_A):
            out[b, a * CHUNK:(a + 1) * CHUNK] = \
                res_a[b]["out"][i * CHUNK:(i + 1) * CHUNK]
        for i, a in enumerate(CHUNKS_B):
            out[b, a * CHUNK:(a + 1) * CHUNK] = \
                res_b[b]["out"][i * CHUNK:(i + 1) * CHUNK]
    return out
